# revision 1
# baseline (speedup 1.0000x reference)
"""3-layer GCN (DGL GraphConv norm='both') on 8 TRN2 NeuronCores via Bass/Tile.

Strategy (graph-parallel, dst-sharded):
  - nodes partitioned across M cores by range; core c owns dst rows [c*NPC,(c+1)*NPC)
  - per layer: each core projects its own node slice (scale by out_isqrt, @ W,
    bf16), AllGather the projected table to every core's DRAM, then aggregate
    its edges: dma_gather source rows by (dst-sorted, src-sorted) edge lists,
    one-hot matmul accumulation in PSUM per 128-dst tile, epilogue
    relu(in_isqrt * agg).
  - int16 gather indices: table split in two halves (A: rows < SPLIT, B: rest).
"""
import numpy as np
import sys

sys.path.insert(0, "/opt/trn_rl_repo")

import concourse.bass as bass
import concourse.mybir as mybir
import concourse.tile as tile
from concourse import bacc

P = 128
F = 128  # feature/hidden width (layers 1,2 in + out; layer 3 padded 64->128)


class Cfg:
    def __init__(self, n_nodes, n_cores):
        assert n_nodes % n_cores == 0
        self.N = n_nodes
        self.M = n_cores
        self.NPC = n_nodes // n_cores
        self.T = -(-self.NPC // P)
        self.NPC_PAD = self.T * P
        self.TBL_ROWS = self.M * self.NPC_PAD
        # A/B split: first half of the ranks (must be <= 32768 rows per side)
        self.SPLIT = (self.M // 2) * self.NPC_PAD
        assert self.SPLIT <= 32768 and self.TBL_ROWS - self.SPLIT <= 32768
        # set by preprocess():
        self.NA = self.NB = self.NBLK = 0

    @property
    def NA16(self):
        return self.NA // 16

    @property
    def NB16(self):
        return self.NB // 16

    @property
    def NA128(self):
        return self.NA // P

    @property
    def NB128(self):
        return self.NB // P


def table_row(cfg, node):
    r = node // cfg.NPC
    return r * cfg.NPC_PAD + (node - r * cfg.NPC)


def preprocess(cfg, edge_index):
    """Per-core gather metadata: wrapped int16 idx arrays + dstlocal slot arrays."""
    src = np.asarray(edge_index[0]).astype(np.int64)
    dst = np.asarray(edge_index[1]).astype(np.int64)
    M, T, NPC = cfg.M, cfg.T, cfg.NPC

    rows = table_row(cfg, src)
    core = dst // NPC
    dloc = dst - core * NPC
    tloc = dloc // P
    slot = dloc - tloc * P

    order = np.lexsort((rows, tloc, core))
    rows_s, tloc_s, slot_s = rows[order], tloc[order], slot[order]
    core_s = core[order]
    isA_s = rows_s < cfg.SPLIT
    key = core_s * T + tloc_s
    bounds = np.searchsorted(key, np.arange(M * T + 1))

    nA = np.zeros((M, T), dtype=np.int64)
    nB = np.zeros((M, T), dtype=np.int64)
    for c in range(M):
        for t in range(T):
            k = c * T + t
            s, e = bounds[k], bounds[k + 1]
            a = isA_s[s:e].sum()
            nA[c, t] = a
            nB[c, t] = (e - s) - a

    cfg.NA = int(-(-max(1, nA.max()) // P) * P)
    cfg.NB = int(-(-max(1, nB.max()) // P) * P)
    cfg.NBLK = (cfg.NA + cfg.NB) // P
    NA, NB, NBLK, NA16, NB16 = cfg.NA, cfg.NB, cfg.NBLK, cfg.NA16, cfg.NB16

    idxA = np.zeros((M, 16, T * NA16), dtype=np.int16)
    idxB = np.zeros((M, 16, T * NB16), dtype=np.int16)
    # (replicated to 128 partitions at the end: one copy per gpsimd core)
    dstloc = np.full((M, P, T * NBLK), -1.0, dtype=np.float32)

    for c in range(M):
        for t in range(T):
            k = c * T + t
            s, e = bounds[k], bounds[k + 1]
            amask = isA_s[s:e]
            ra, sa = rows_s[s:e][amask], slot_s[s:e][amask]
            rb, sb = rows_s[s:e][~amask] - cfg.SPLIT, slot_s[s:e][~amask]
            fa = np.zeros(NA, dtype=np.int64)
            fa[: len(ra)] = ra
            fb = np.zeros(NB, dtype=np.int64)
            fb[: len(rb)] = rb
            ga = np.full(NA, -1.0, dtype=np.float32)
            ga[: len(sa)] = sa
            gb = np.full(NB, -1.0, dtype=np.float32)
            gb[: len(sb)] = sb
            idxA[c, :, t * NA16:(t + 1) * NA16] = fa.reshape(NA16, 16).T
            idxB[c, :, t * NB16:(t + 1) * NB16] = fb.reshape(NB16, 16).T
            g = np.concatenate([ga, gb])
            dstloc[c, :, t * NBLK:(t + 1) * NBLK] = g.reshape(NBLK, P).T

    return np.tile(idxA, (1, 8, 1)), np.tile(idxB, (1, 8, 1)), dstloc


def pack_percore(cfg, vec):
    """[N] f32 -> [M][128, T]: node c*NPC + t*128 + p -> [c, p, t] (pad 1.0)."""
    out = np.ones((cfg.M, P, cfg.T), dtype=np.float32)
    for c in range(cfg.M):
        v = np.ones(cfg.NPC_PAD, dtype=np.float32)
        v[: cfg.NPC] = vec[c * cfg.NPC:(c + 1) * cfg.NPC]
        out[c] = v.reshape(cfg.T, P).T
    return out


def make_inputs(cfg, h, edge_index, W1, W2, W3):
    """Host preprocessing -> in_maps list for run_bass_kernel_spmd."""
    h = np.asarray(h, dtype=np.float32)
    idxA, idxB, dstloc = preprocess(cfg, edge_index)
    src = np.asarray(edge_index[0])
    dst = np.asarray(edge_index[1])
    out_deg = np.bincount(src, minlength=cfg.N).astype(np.float32)
    in_deg = np.bincount(dst, minlength=cfg.N).astype(np.float32)
    oi = pack_percore(cfg, np.clip(out_deg, 1.0, None) ** -0.5)
    ii = pack_percore(cfg, np.clip(in_deg, 1.0, None) ** -0.5)

    W3p = np.zeros((F, F), dtype=np.float32)
    W3p[:, : np.asarray(W3).shape[1]] = np.asarray(W3)
    Ws = np.stack([np.asarray(W1, np.float32), np.asarray(W2, np.float32), W3p])

    in_maps = []
    for c in range(cfg.M):
        hc = np.zeros((cfg.NPC_PAD, F), dtype=np.float32)
        hc[: cfg.NPC] = h[c * cfg.NPC:(c + 1) * cfg.NPC]
        in_maps.append(
            dict(
                h=hc,
                ws=Ws,
                idxA=np.ascontiguousarray(idxA[c]),
                idxB=np.ascontiguousarray(idxB[c]),
                dstloc=np.ascontiguousarray(dstloc[c]),
                oi=np.ascontiguousarray(oi[c]),
                ii=np.ascontiguousarray(ii[c]),
            )
        )
    return in_maps


def bcast(ap, new_ap):
    return bass.AP(ap.tensor, ap.offset, new_ap)


def build_program(cfg, n_chunk=4, max_gather=16 * 1024, shared_tables=True,
                 dma_scratch=16384):
    f32, bf16, i16 = mybir.dt.float32, mybir.dt.bfloat16, mybir.dt.int16
    T, NA, NB, NBLK = cfg.T, cfg.NA, cfg.NB, cfg.NBLK
    NA16, NB16, NA128, NB128 = cfg.NA16, cfg.NB16, cfg.NA128, cfg.NB128

    nc = bacc.Bacc(
        "TRN2",
        target_bir_lowering=False,
        debug=False,
        num_devices=cfg.M,
        num_swdge_queues=1,
        dynamic_dma_scratch_size=dma_scratch,
    )

    h_d = nc.dram_tensor("h", [cfg.NPC_PAD, F], f32, kind="ExternalInput")
    ws_d = nc.dram_tensor("ws", [3, F, F], f32, kind="ExternalInput")
    idxA_d = nc.dram_tensor("idxA", [P, T * NA16], i16, kind="ExternalInput")
    idxB_d = nc.dram_tensor("idxB", [P, T * NB16], i16, kind="ExternalInput")
    dstloc_d = nc.dram_tensor("dstloc", [P, T * NBLK], f32, kind="ExternalInput")
    oi_d = nc.dram_tensor("oi", [P, T], f32, kind="ExternalInput")
    ii_d = nc.dram_tensor("ii", [P, T], f32, kind="ExternalInput")
    out_d = nc.dram_tensor("out", [cfg.NPC_PAD, 64], f32, kind="ExternalOutput")

    chunks = []
    t0 = 0
    while t0 < T:
        ct = min(n_chunk, T - t0)
        chunks.append((t0, ct))
        t0 += ct

    with tile.TileContext(nc) as tc:
        with tc.tile_pool(name="const", bufs=1) as cpool, \
             tc.tile_pool(name="dram", bufs=1, space="DRAM") as dpool, \
             tc.tile_pool(name="work", bufs=3) as wpool, \
             tc.tile_pool(name="gpoolA", bufs=2) as gpoolA, \
             tc.tile_pool(name="gpoolB", bufs=2) as gpoolB, \
             tc.tile_pool(name="ohpool", bufs=2) as ohpool, \
             tc.tile_pool(name="psA", bufs=3, space="PSUM") as psA, \
             tc.tile_pool(name="psP", bufs=2, space="PSUM") as psP:

            # ---------- resident constants ----------
            from concourse.masks import make_identity

            ident = cpool.tile([P, P], bf16)
            make_identity(nc, ident[:])

            iota_i = cpool.tile([P, P], i16)
            nc.gpsimd.iota(iota_i[:], pattern=[[1, P]], base=0, channel_multiplier=0)
            iota_b = cpool.tile([P, P], bf16)
            nc.vector.tensor_copy(iota_b[:], iota_i[:])

            idxA_sb = cpool.tile([P, T * NA16], i16)
            nc.sync.dma_start(idxA_sb[:], idxA_d[:, :])
            idxB_sb = cpool.tile([P, T * NB16], i16)
            nc.sync.dma_start(idxB_sb[:], idxB_d[:, :])

            dstf = wpool.tile([P, T * NBLK], f32, tag="dstf", bufs=1)
            nc.sync.dma_start(dstf[:], dstloc_d[:, :])
            dstb = cpool.tile([P, T * NBLK], bf16)
            nc.vector.tensor_copy(dstb[:], dstf[:])

            oi_sb = cpool.tile([P, T], f32)
            nc.sync.dma_start(oi_sb[:], oi_d[:, :])
            ii_sb = cpool.tile([P, T], f32)
            nc.sync.dma_start(ii_sb[:], ii_d[:, :])

            w_sb = []
            for l in range(3):
                wf = wpool.tile([P, F], f32, tag="wf", bufs=1)
                nc.sync.dma_start(wf[:], ws_d[l])
                wb = cpool.tile([P, F], bf16, name=f"w{l}")
                nc.vector.tensor_copy(wb[:], wf[:])
                w_sb.append(wb)

            hping = cpool.tile([P, T, F], bf16)
            hpong = cpool.tile([P, T, F], bf16)

            pslice = dpool.tile([cfg.NPC_PAD, F], bf16)
            ptables = [
                dpool.tile([cfg.TBL_ROWS, F], bf16,
                           addr_space="Shared" if shared_tables else "Local",
                           name=f"ptable{i}")
                for i in range(3)
            ]

            def emit_gathers(gbuf, idx_sb, n16pt, total, t0, src_ap, qbase):
                # Single SWDGE queue: Tile assigns DMASW completion sems
                # round-robin in *scheduled* order while sems are locked to
                # one queue each — multi-queue only works if queue_num
                # matches that order, which we can't control. Queue 0 always
                # satisfies the lock.
                o = 0
                while o < total:
                    ln = min(max_gather, total - o)
                    nc.gpsimd.dma_gather(
                        out_ap=gbuf[:, o // 128:(o + ln) // 128, :],
                        in_ap=src_ap,
                        idxs_ap=idx_sb[:, t0 * n16pt + o // 16:
                                       t0 * n16pt + (o + ln) // 16],
                        num_idxs=ln,
                        num_idxs_reg=ln,
                        elem_size=F,
                        queue_num=0,
                    )
                    o += ln

            for l in range(3):
                ptable = ptables[l]
                hcur = [None, hping, hpong][l]
                hnext = [hping, hpong, None][l]

                # ---------- projection of own slice ----------
                for t in range(T):
                    hs = wpool.tile([P, F], bf16, tag="hs")
                    if l == 0:
                        h0t = wpool.tile([P, F], f32, tag="h0t")
                        nc.sync.dma_start(h0t[:], h_d[t * P:(t + 1) * P, :])
                        nc.vector.tensor_scalar_mul(hs[:], h0t[:], oi_sb[:, t:t + 1])
                    else:
                        nc.vector.tensor_scalar_mul(hs[:], hcur[:, t, :], oi_sb[:, t:t + 1])
                    tp = psP.tile([P, P], bf16, space="PSUM", tag="tp")
                    nc.tensor.transpose(tp[:], hs[:], ident[:])
                    hsT = wpool.tile([P, P], bf16, tag="hsT")
                    nc.vector.tensor_copy(hsT[:], tp[:])
                    pp = psP.tile([P, F], f32, space="PSUM", tag="pp")
                    nc.tensor.matmul(pp[:], lhsT=hsT[:], rhs=w_sb[l][:], start=True, stop=True)
                    pout = wpool.tile([P, F], bf16, tag="pout")
                    nc.vector.tensor_copy(pout[:], pp[:])
                    nc.sync.dma_start(pslice[t * P:(t + 1) * P, :], pout[:])

                # ---------- all-gather the projected table ----------
                nc.gpsimd.collective_compute(
                    "AllGather",
                    mybir.AluOpType.bypass,
                    replica_groups=[list(range(cfg.M))],
                    ins=[pslice.opt()],
                    outs=[ptable.opt()],
                )

                # ---------- aggregation ----------
                for ci, (t0, ct) in enumerate(chunks):
                    gA = gpoolA.tile([P, ct * NA128, F], bf16, tag="gA")
                    emit_gathers(gA, idxA_sb, NA16, ct * NA, t0,
                                 ptable[:], 2 * ci)
                    gB = gpoolB.tile([P, ct * NB128, F], bf16, tag="gB")
                    emit_gathers(gB, idxB_sb, NB16, ct * NB, t0,
                                 ptable[cfg.SPLIT:, :], 2 * ci + 1)
                    for tt in range(ct):
                        t = t0 + tt
                        oh = ohpool.tile([P, NBLK, P], bf16, tag="oh")
                        i0 = iota_b[:]
                        d0 = dstb[:, t * NBLK:(t + 1) * NBLK]
                        nc.vector.tensor_tensor(
                            out=oh[:],
                            in0=bcast(i0, [i0.ap[0], [0, NBLK], [1, P]]),
                            in1=bcast(d0, [d0.ap[0], [1, NBLK], [0, P]]),
                            op=mybir.AluOpType.is_equal,
                        )
                        ap_ps = psA.tile([P, F], f32, space="PSUM", tag="ap_ps")
                        for b in range(NA128):
                            nc.tensor.matmul(
                                ap_ps[:],
                                lhsT=oh[:, b, :],
                                rhs=gA[:, tt * NA128 + b, :],
                                start=(b == 0),
                                stop=False,
                            )
                        for b in range(NB128):
                            nc.tensor.matmul(
                                ap_ps[:],
                                lhsT=oh[:, NA128 + b, :],
                                rhs=gB[:, tt * NB128 + b, :],
                                start=False,
                                stop=(b == NB128 - 1),
                            )
                        if l < 2:
                            nc.vector.tensor_scalar(
                                out=hnext[:, t, :],
                                in0=ap_ps[:],
                                scalar1=ii_sb[:, t:t + 1],
                                scalar2=0.0,
                                op0=mybir.AluOpType.mult,
                                op1=mybir.AluOpType.max,
                            )
                        else:
                            ot = wpool.tile([P, 64], f32, tag="ot")
                            nc.vector.tensor_scalar(
                                out=ot[:],
                                in0=ap_ps[:, :64],
                                scalar1=ii_sb[:, t:t + 1],
                                scalar2=0.0,
                                op0=mybir.AluOpType.mult,
                                op1=mybir.AluOpType.max,
                            )
                            nc.sync.dma_start(out_d[t * P:(t + 1) * P, :], ot[:])
    return nc


def run(cfg, h, edge_index, W1, W2, W3, n_chunk=4, trace=False,
        max_gather=16 * 1024, shared_tables=True, dma_scratch=16384, **kw):
    from concourse.bass_utils import run_bass_kernel_spmd

    import os
    import time

    in_maps = make_inputs(cfg, h, edge_index, W1, W2, W3)
    nc = build_program(cfg, n_chunk=n_chunk, max_gather=max_gather,
                       shared_tables=shared_tables, dma_scratch=dma_scratch)
    if not nc.is_finalized():
        nc.finalize()
    res = run_bass_kernel_spmd(
        nc, in_maps, core_ids=list(range(cfg.M)), trace=trace, **kw
    )
    if os.environ.get("GCN_BENCH", "0") == "1":
        times = []
        for _ in range(3):
            t0 = time.time()
            res = run_bass_kernel_spmd(
                nc, in_maps, core_ids=list(range(cfg.M)), trace=trace, **kw
            )
            times.append(time.time() - t0)
        print(f"bench walls: {[f'{t:.2f}' for t in times]} s (incl upload/retrace)")
    outs = [res.results[c]["out"][: cfg.NPC, :] for c in range(cfg.M)]
    return np.concatenate(outs, axis=0), res


# ----------------------------------------------------------------------------
# Harness entry point: full (unsharded) inputs -> full output.
# ----------------------------------------------------------------------------
LAST_EXEC_NS = None


def kernel(h, edge_index, W1, W2, W3):
    import os

    global LAST_EXEC_NS
    cfg = Cfg(50000, 8)
    trace = bool(int(os.environ.get("GCN_TRACE", "0")))
    max_gather = int(os.environ.get("GCN_MAX_GATHER", "768"))
    shared = os.environ.get("GCN_SHARED", "0") == "1"
    out, res = run(cfg, h, edge_index, W1, W2, W3, n_chunk=4, trace=trace,
                   max_gather=max_gather, shared_tables=shared)
    LAST_EXEC_NS = res.exec_time_ns
    return np.ascontiguousarray(out, dtype=np.float32)



# revision 23
# speedup vs baseline: 2.6444x; 2.6444x over previous
"""3-layer GCN (DGL GraphConv norm='both') on 8 TRN2 NeuronCores via Bass/Tile.

Strategy (graph-parallel, dst-sharded):
  - nodes partitioned across M cores by range; core c owns dst rows [c*NPC,(c+1)*NPC)
  - per layer: each core projects its own node slice (scale by out_isqrt, @ W,
    bf16), AllGather the projected table to every core's DRAM, then aggregate
    its edges: dma_gather source rows by (dst-sorted, src-sorted) edge lists,
    one-hot matmul accumulation in PSUM per 128-dst tile, epilogue
    relu(in_isqrt * agg).
  - int16 gather indices: table split in two halves (A: rows < SPLIT, B: rest).
"""
import numpy as np
import sys

sys.path.insert(0, "/opt/trn_rl_repo")

import concourse.bass as bass
import concourse.mybir as mybir
import concourse.tile as tile
from concourse import bacc

P = 128
F = 128  # feature/hidden width (layers 1,2 in + out; layer 3 padded 64->128)


class Cfg:
    def __init__(self, n_nodes, n_cores):
        assert n_nodes % n_cores == 0
        self.N = n_nodes
        self.M = n_cores
        self.NPC = n_nodes // n_cores
        self.T = -(-self.NPC // P)
        self.NPC_PAD = self.T * P
        self.TBL_ROWS = self.M * self.NPC_PAD
        # A/B split: first half of the ranks (must be <= 32768 rows per side)
        self.SPLIT = (self.M // 2) * self.NPC_PAD
        assert self.SPLIT <= 32768 and self.TBL_ROWS - self.SPLIT <= 32768
        # set by preprocess():
        self.NA = self.NB = self.NBLK = 0

    @property
    def NA16(self):
        return self.NA // 16

    @property
    def NB16(self):
        return self.NB // 16

    @property
    def NA128(self):
        return self.NA // P

    @property
    def NB128(self):
        return self.NB // P


def table_row(cfg, node):
    r = node // cfg.NPC
    return r * cfg.NPC_PAD + (node - r * cfg.NPC)


def preprocess(cfg, edge_index, full_pad_tiles=8):
    """Per-core gather metadata: wrapped int16 idx arrays + dstlocal slot arrays."""
    src = np.asarray(edge_index[0]).astype(np.int64)
    dst = np.asarray(edge_index[1]).astype(np.int64)
    M, T, NPC = cfg.M, cfg.T, cfg.NPC

    rows = table_row(cfg, src)
    core = dst // NPC
    dloc = dst - core * NPC
    tloc = dloc // P
    slot = dloc - tloc * P

    order = np.lexsort((rows, tloc, core))
    rows_s, tloc_s, slot_s = rows[order], tloc[order], slot[order]
    core_s = core[order]
    isA_s = rows_s < cfg.SPLIT
    key = core_s * T + tloc_s
    bounds = np.searchsorted(key, np.arange(M * T + 1))

    nA = np.zeros((M, T), dtype=np.int64)
    nB = np.zeros((M, T), dtype=np.int64)
    for c in range(M):
        for t in range(T):
            k = c * T + t
            s, e = bounds[k], bounds[k + 1]
            a = isA_s[s:e].sum()
            nA[c, t] = a
            nB[c, t] = (e - s) - a

    cfg.NA = int(-(-max(1, nA.max()) // P) * P)
    cfg.NB = int(-(-max(1, nB.max()) // P) * P)
    cfg.NBLK = (cfg.NA + cfg.NB) // P
    NA, NB, NBLK, NA16, NB16 = cfg.NA, cfg.NB, cfg.NBLK, cfg.NA16, cfg.NB16
    cfg.nA, cfg.nB = nA, nB

    idxA = np.zeros((M, 16, T * NA16), dtype=np.int16)
    idxB = np.zeros((M, 16, T * NB16), dtype=np.int16)
    # (replicated to 128 partitions at the end: one copy per gpsimd core)
    dstloc = np.full((M, P, T * NBLK), -1.0, dtype=np.float32)

    for c in range(M):
        for t in range(T):
            k = c * T + t
            s, e = bounds[k], bounds[k + 1]
            amask = isA_s[s:e]
            ra, sa = rows_s[s:e][amask], slot_s[s:e][amask]
            rb, sb = rows_s[s:e][~amask] - cfg.SPLIT, slot_s[s:e][~amask]
            # -1 padding: the gather ucode trims trailing negative idxs
            # before descriptor generation (saves Pool desc-gen time).
            # Exception: tiles in the first two chunks (first use of each
            # gather pool slot) pad with row 0 so every slot position gets
            # written at least once — uninitialized SBUF could decode as
            # NaN/Inf in bf16 and 0 * NaN would pollute the one-hot matmul.
            pad = 0 if t < full_pad_tiles else -1
            fa = np.full(NA, pad, dtype=np.int64)
            fa[: len(ra)] = ra
            fb = np.full(NB, pad, dtype=np.int64)
            fb[: len(rb)] = rb
            ga = np.full(NA, -1.0, dtype=np.float32)
            ga[: len(sa)] = sa
            gb = np.full(NB, -1.0, dtype=np.float32)
            gb[: len(sb)] = sb
            idxA[c, :, t * NA16:(t + 1) * NA16] = fa.reshape(NA16, 16).T
            idxB[c, :, t * NB16:(t + 1) * NB16] = fb.reshape(NB16, 16).T
            g = np.concatenate([ga, gb])
            dstloc[c, :, t * NBLK:(t + 1) * NBLK] = g.reshape(NBLK, P).T

    return np.tile(idxA, (1, 8, 1)), np.tile(idxB, (1, 8, 1)), dstloc


def pack_percore(cfg, vec):
    """[N] f32 -> [M][128, T]: node c*NPC + t*128 + p -> [c, p, t] (pad 1.0)."""
    out = np.ones((cfg.M, P, cfg.T), dtype=np.float32)
    for c in range(cfg.M):
        v = np.ones(cfg.NPC_PAD, dtype=np.float32)
        v[: cfg.NPC] = vec[c * cfg.NPC:(c + 1) * cfg.NPC]
        out[c] = v.reshape(cfg.T, P).T
    return out


def make_inputs(cfg, h, edge_index, W1, W2, W3, full_pad_tiles=8,
                max_gather=768):
    """Host preprocessing -> in_maps list for run_bass_kernel_spmd."""
    h = np.asarray(h, dtype=np.float32)
    idxA, idxB, dstloc = preprocess(cfg, edge_index, full_pad_tiles)

    # Last-call valid counts per (tile, half): only the final <=max_gather
    # window of each tile can be partial; its exact count feeds the gather's
    # num_idxs_reg (the ring bookkeeping needs reg == trimmed desc count).
    ka = -(-cfg.NA // max_gather)
    kb = -(-cfg.NB // max_gather)
    # Tiles padded with row 0 (t < full_pad_tiles) are fully valid.
    nA_eff = cfg.nA.copy()
    nB_eff = cfg.nB.copy()
    nA_eff[:, :full_pad_tiles] = cfg.NA
    nB_eff[:, :full_pad_tiles] = cfg.NB
    assert (nA_eff >= (ka - 1) * max_gather).all(), "non-final A call partial"
    assert (nB_eff >= (kb - 1) * max_gather).all(), "non-final B call partial"
    gcnt = np.zeros((cfg.M, P, 2 * cfg.T), dtype=np.int32)
    gcnt[:, :, 0::2] = (nA_eff - (ka - 1) * max_gather)[:, None, :]
    gcnt[:, :, 1::2] = (nB_eff - (kb - 1) * max_gather)[:, None, :]
    src = np.asarray(edge_index[0])
    dst = np.asarray(edge_index[1])
    out_deg = np.bincount(src, minlength=cfg.N).astype(np.float32)
    in_deg = np.bincount(dst, minlength=cfg.N).astype(np.float32)
    oi = pack_percore(cfg, np.clip(out_deg, 1.0, None) ** -0.5)
    ii = pack_percore(cfg, np.clip(in_deg, 1.0, None) ** -0.5)

    W3p = np.zeros((F, F), dtype=np.float32)
    W3p[:, : np.asarray(W3).shape[1]] = np.asarray(W3)
    Ws = np.stack([np.asarray(W1, np.float32), np.asarray(W2, np.float32), W3p])

    in_maps = []
    for c in range(cfg.M):
        hc = np.zeros((cfg.NPC_PAD, F), dtype=np.float32)
        hc[: cfg.NPC] = h[c * cfg.NPC:(c + 1) * cfg.NPC]
        in_maps.append(
            dict(
                h=hc,
                ws=Ws,
                idxA=np.ascontiguousarray(idxA[c]),
                idxB=np.ascontiguousarray(idxB[c]),
                dstloc=np.ascontiguousarray(dstloc[c]),
                oi=np.ascontiguousarray(oi[c]),
                ii=np.ascontiguousarray(ii[c]),
                gcnt=np.ascontiguousarray(gcnt[c]),
            )
        )
    return in_maps


def bcast(ap, new_ap):
    return bass.AP(ap.tensor, ap.offset, new_ap)


def build_program(cfg, n_chunk=4, max_gather=16 * 1024, shared_tables=True,
                 dma_scratch=16384, nqueues=1):
    f32, bf16, i16 = mybir.dt.float32, mybir.dt.bfloat16, mybir.dt.int16
    T, NA, NB, NBLK = cfg.T, cfg.NA, cfg.NB, cfg.NBLK
    NA16, NB16, NA128, NB128 = cfg.NA16, cfg.NB16, cfg.NA128, cfg.NB128

    nc = bacc.Bacc(
        "TRN2",
        target_bir_lowering=False,
        debug=False,
        num_devices=cfg.M,
        num_swdge_queues=nqueues,
        dynamic_dma_scratch_size=dma_scratch,
    )
    # Global SWDGE-call counter: queue = (call# mod 8) // 2 keeps the
    # queue↔DMASW-lane map consistent when Tile assigns DMASW lanes
    # round-robin in scheduled order (assuming emission order is kept).
    qctr = [0]

    h_d = nc.dram_tensor("h", [cfg.NPC_PAD, F], f32, kind="ExternalInput")
    ws_d = nc.dram_tensor("ws", [3, F, F], f32, kind="ExternalInput")
    idxA_d = nc.dram_tensor("idxA", [P, T * NA16], i16, kind="ExternalInput")
    idxB_d = nc.dram_tensor("idxB", [P, T * NB16], i16, kind="ExternalInput")
    dstloc_d = nc.dram_tensor("dstloc", [P, T * NBLK], f32, kind="ExternalInput")
    oi_d = nc.dram_tensor("oi", [P, T], f32, kind="ExternalInput")
    ii_d = nc.dram_tensor("ii", [P, T], f32, kind="ExternalInput")
    gcnt_d = nc.dram_tensor("gcnt", [P, 2 * T], mybir.dt.int32,
                            kind="ExternalInput")
    out_d = nc.dram_tensor("out", [cfg.NPC_PAD, 64], f32, kind="ExternalOutput")

    chunks = []
    t0 = 0
    while t0 < T:
        ct = min(n_chunk, T - t0)
        chunks.append((t0, ct))
        t0 += ct

    with tile.TileContext(nc) as tc:
        with tc.tile_pool(name="const", bufs=1) as cpool, \
             tc.tile_pool(name="dram", bufs=1, space="DRAM") as dpool, \
             tc.tile_pool(name="work", bufs=3) as wpool, \
             tc.tile_pool(name="gpoolA", bufs=2) as gpoolA, \
             tc.tile_pool(name="gpoolB", bufs=2) as gpoolB, \
             tc.tile_pool(name="ohpool", bufs=2) as ohpool, \
             tc.tile_pool(name="psA", bufs=3, space="PSUM") as psA, \
             tc.tile_pool(name="psP", bufs=2, space="PSUM") as psP:

            # ---------- resident constants ----------
            from concourse.masks import make_identity

            ident = cpool.tile([P, P], bf16)
            make_identity(nc, ident[:])

            iota_i = cpool.tile([P, P], i16)
            nc.gpsimd.iota(iota_i[:], pattern=[[1, P]], base=0, channel_multiplier=0)
            iota_b = cpool.tile([P, P], bf16)
            nc.vector.tensor_copy(iota_b[:], iota_i[:])

            idxA_sb = cpool.tile([P, T * NA16], i16)
            nc.sync.dma_start(idxA_sb[:], idxA_d[:, :])
            idxB_sb = cpool.tile([P, T * NB16], i16)
            nc.sync.dma_start(idxB_sb[:], idxB_d[:, :])

            dstf = wpool.tile([P, T * NBLK], f32, tag="dstf", bufs=1)
            nc.sync.dma_start(dstf[:], dstloc_d[:, :])
            dstb = cpool.tile([P, T * NBLK], bf16)
            nc.vector.tensor_copy(dstb[:], dstf[:])

            oi_sb = cpool.tile([P, T], f32)
            nc.sync.dma_start(oi_sb[:], oi_d[:, :])
            ii_sb = cpool.tile([P, T], f32)
            nc.sync.dma_start(ii_sb[:], ii_d[:, :])
            gcnt_sb = cpool.tile([P, 2 * T], mybir.dt.int32)
            nc.sync.dma_start(gcnt_sb[:], gcnt_d[:, :])
            greg = [nc.gpsimd.alloc_register(f"gcnt_r{i}") for i in range(2)]

            w_sb = []
            for l in range(3):
                wf = wpool.tile([P, F], f32, tag="wf", bufs=1)
                nc.sync.dma_start(wf[:], ws_d[l])
                wb = cpool.tile([P, F], bf16, name=f"w{l}")
                nc.vector.tensor_copy(wb[:], wf[:])
                w_sb.append(wb)

            hping = cpool.tile([P, T, F], bf16)
            hpong = cpool.tile([P, T, F], bf16)

            pslice = dpool.tile([cfg.NPC_PAD, F], bf16)
            ptables = [
                dpool.tile([cfg.TBL_ROWS, F], bf16,
                           addr_space="Shared" if shared_tables else "Local",
                           name=f"ptable{i}")
                for i in range(3)
            ]

            def emit_gathers(gbuf, idx_sb, n16pt, per_tile, ct, t0, src_ap,
                             half):
                # Call windows are aligned to (tile, half) boundaries so the
                # trailing -1 padding of the FINAL window per tile is trimmed
                # by the gather ucode before descriptor generation (Pool
                # desc-gen is the bottleneck at ~7.5ns/idx). num_idxs_reg
                # must equal the exact trimmed count or the decode-stage ring
                # bookkeeping diverges from the pushed descriptors (hang), so
                # the final window's count is loaded into a Pool register.
                for tt in range(ct):
                    for oo in range(0, per_tile, max_gather):
                        o = tt * per_tile + oo
                        ln = min(max_gather, per_tile - oo)
                        q = (qctr[0] % 8) // 2 % nqueues if nqueues > 1 else 0
                        qctr[0] += 1
                        last = oo + max_gather >= per_tile
                        if last:
                            reg = greg[half]
                            nc.gpsimd.reg_load(
                                reg,
                                gcnt_sb[0:1, 2 * (t0 + tt) + half:
                                        2 * (t0 + tt) + half + 1])
                            nreg = reg
                        else:
                            nreg = ln
                        nc.gpsimd.dma_gather(
                            out_ap=gbuf[:, o // 128:(o + ln) // 128, :],
                            in_ap=src_ap,
                            idxs_ap=idx_sb[:, t0 * n16pt + o // 16:
                                           t0 * n16pt + (o + ln) // 16],
                            num_idxs=ln,
                            num_idxs_reg=nreg,
                            elem_size=F,
                            queue_num=q,
                        )

            for l in range(3):
                ptable = ptables[l]
                hcur = [None, hping, hpong][l]
                hnext = [hping, hpong, None][l]

                # ---------- projection of own slice ----------
                for t in range(T):
                    hs = wpool.tile([P, F], bf16, tag="hs")
                    if l == 0:
                        h0t = wpool.tile([P, F], f32, tag="h0t")
                        nc.sync.dma_start(h0t[:], h_d[t * P:(t + 1) * P, :])
                        nc.vector.tensor_scalar_mul(hs[:], h0t[:], oi_sb[:, t:t + 1])
                    else:
                        nc.vector.tensor_scalar_mul(hs[:], hcur[:, t, :], oi_sb[:, t:t + 1])
                    tp = psP.tile([P, P], bf16, space="PSUM", tag="tp")
                    nc.tensor.transpose(tp[:], hs[:], ident[:])
                    hsT = wpool.tile([P, P], bf16, tag="hsT")
                    nc.vector.tensor_copy(hsT[:], tp[:])
                    pp = psP.tile([P, F], f32, space="PSUM", tag="pp")
                    nc.tensor.matmul(pp[:], lhsT=hsT[:], rhs=w_sb[l][:], start=True, stop=True)
                    pout = wpool.tile([P, F], bf16, tag="pout")
                    nc.vector.tensor_copy(pout[:], pp[:])
                    nc.sync.dma_start(pslice[t * P:(t + 1) * P, :], pout[:])

                # ---------- all-gather the projected table ----------
                nc.gpsimd.collective_compute(
                    "AllGather",
                    mybir.AluOpType.bypass,
                    replica_groups=[list(range(cfg.M))],
                    ins=[pslice.opt()],
                    outs=[ptable.opt()],
                )

                # ---------- aggregation ----------
                for ci, (t0, ct) in enumerate(chunks):
                    gA = gpoolA.tile([P, ct * NA128, F], bf16, tag="gA")
                    emit_gathers(gA, idxA_sb, NA16, NA, ct, t0, ptable[:], 0)
                    gB = gpoolB.tile([P, ct * NB128, F], bf16, tag="gB")
                    emit_gathers(gB, idxB_sb, NB16, NB, ct, t0,
                                 ptable[cfg.SPLIT:, :], 1)
                    for tt in range(ct):
                        t = t0 + tt
                        oh = ohpool.tile([P, NBLK, P], bf16, tag="oh")
                        i0 = iota_b[:]
                        d0 = dstb[:, t * NBLK:(t + 1) * NBLK]
                        nc.vector.tensor_tensor(
                            out=oh[:],
                            in0=bcast(i0, [i0.ap[0], [0, NBLK], [1, P]]),
                            in1=bcast(d0, [d0.ap[0], [1, NBLK], [0, P]]),
                            op=mybir.AluOpType.is_equal,
                        )
                        ap_ps = psA.tile([P, F], f32, space="PSUM", tag="ap_ps")
                        for b in range(NA128):
                            nc.tensor.matmul(
                                ap_ps[:],
                                lhsT=oh[:, b, :],
                                rhs=gA[:, tt * NA128 + b, :],
                                start=(b == 0),
                                stop=False,
                            )
                        for b in range(NB128):
                            nc.tensor.matmul(
                                ap_ps[:],
                                lhsT=oh[:, NA128 + b, :],
                                rhs=gB[:, tt * NB128 + b, :],
                                start=False,
                                stop=(b == NB128 - 1),
                            )
                        if l < 2:
                            nc.vector.tensor_scalar(
                                out=hnext[:, t, :],
                                in0=ap_ps[:],
                                scalar1=ii_sb[:, t:t + 1],
                                scalar2=0.0,
                                op0=mybir.AluOpType.mult,
                                op1=mybir.AluOpType.max,
                            )
                        else:
                            ot = wpool.tile([P, 64], f32, tag="ot")
                            nc.vector.tensor_scalar(
                                out=ot[:],
                                in0=ap_ps[:, :64],
                                scalar1=ii_sb[:, t:t + 1],
                                scalar2=0.0,
                                op0=mybir.AluOpType.mult,
                                op1=mybir.AluOpType.max,
                            )
                            nc.sync.dma_start(out_d[t * P:(t + 1) * P, :], ot[:])
    return nc


def run(cfg, h, edge_index, W1, W2, W3, n_chunk=4, trace=False,
        max_gather=16 * 1024, shared_tables=True, dma_scratch=16384,
        nqueues=1, **kw):
    from concourse.bass_utils import run_bass_kernel_spmd

    import os
    import time

    in_maps = make_inputs(cfg, h, edge_index, W1, W2, W3,
                          full_pad_tiles=2 * n_chunk, max_gather=max_gather)
    nc = build_program(cfg, n_chunk=n_chunk, max_gather=max_gather,
                       shared_tables=shared_tables, dma_scratch=dma_scratch,
                       nqueues=nqueues)
    if not nc.is_finalized():
        nc.finalize()
    res = run_bass_kernel_spmd(
        nc, in_maps, core_ids=list(range(cfg.M)), trace=trace, **kw
    )
    if os.environ.get("GCN_BENCH", "0") == "1":
        times = []
        for _ in range(3):
            t0 = time.time()
            res = run_bass_kernel_spmd(
                nc, in_maps, core_ids=list(range(cfg.M)), trace=trace, **kw
            )
            times.append(time.time() - t0)
        print(f"bench walls: {[f'{t:.2f}' for t in times]} s (incl upload/retrace)")
    outs = [res.results[c]["out"][: cfg.NPC, :] for c in range(cfg.M)]
    return np.concatenate(outs, axis=0), res


# ----------------------------------------------------------------------------
# Harness entry point: full (unsharded) inputs -> full output.
# ----------------------------------------------------------------------------
LAST_EXEC_NS = None


def kernel(h, edge_index, W1, W2, W3):
    import os

    global LAST_EXEC_NS
    cfg = Cfg(50000, 8)
    trace = bool(int(os.environ.get("GCN_TRACE", "0")))
    max_gather = int(os.environ.get("GCN_MAX_GATHER", "768"))
    shared = os.environ.get("GCN_SHARED", "1") == "1"
    nqueues = int(os.environ.get("GCN_QUEUES", "1"))
    out, res = run(cfg, h, edge_index, W1, W2, W3, n_chunk=4, trace=trace,
                   max_gather=max_gather, shared_tables=shared,
                   nqueues=nqueues)
    LAST_EXEC_NS = res.exec_time_ns
    return np.ascontiguousarray(out, dtype=np.float32)

